# revision 23
# baseline (speedup 1.0000x reference)
"""Trainium2 Bass kernel for nn_Attention_49263274885969 (v6).

The reference returns only out[:, 0, :] — the attention output of the single
prepended tmp_token row. Exploiting linearity, the whole module collapses to,
per batch b (x_full = [tmp_token; x_b], [2049, 1024]):

    scores[n, h] = x_full[n, :] @ w[:, h]        w = (Wk_h @ q0_h) * Dh^-0.5
    att          = exp(scores); den[h] = sum_n att + e0[h]   (host e0)
    r[h, :]      = ((att[:, h] @ x_full) + e0[h]*tmp) / den[h]
    o            = concat_h(r[h] @ Wvp[:, h*256:(h+1)*256])
    out_b        = o @ Wvf + (bvp @ Wvf + bvf)   (bias const added on host)

All matmuls are arranged "operand-swapped" (large operand stationary, 4-wide
moving operand) so every intermediate lands already transposed and no PE
transposes are needed. vs the 41.3us fp16 baseline: the four large tensors
(xT, xN, 16*Wvp, 16*Wvf) are fp8 e3m4, halving per-core DMA 12.6 -> 6.3 MB;
att/r/o and the score weights stay fp16 (mixed-dtype matmuls are legal).
The x16 weight scalings (to reach e3m4 normal range) are compensated exactly
by folding 1/256 into the inv-denominator broadcast row (fp32 psum, so no
fp16 subnormal loss). Measured rel err ~1.1e-2 < 2e-2 gate.
Data-parallel: one batch per NeuronCore, 8 cores, full inputs in, full out.
"""

import numpy as np
import ml_dtypes
from contextlib import ExitStack

import concourse.bass as bass
from concourse import bacc
import concourse.mybir as mybir
import concourse.tile as tile
from concourse.bass_utils import run_bass_kernel_spmd
from concourse.tile_rust import add_dep_helper

F16 = np.float16
E3 = ml_dtypes.float8_e3m4
P = 128
B, N, C = 8, 2048, 1024
H, Dh = 4, 256
TCH = 16
CCH = C // P
WS = 16.0             # host scale on Wvp/Wvf (e3m4 normal range)
XSLAB = 8192          # cols per xT/xN DMA slab
WSLAB = 2048          # cols per wvp/wvf DMA slab

LAST_RESULTS = None
_NC_CACHE = {}


def _build_bass(reps=1):
    nc = bacc.Bacc("TRN2", debug=False)
    fp32 = mybir.dt.float32
    fp16 = mybir.dt.float16
    fp8 = mybir.dt.float8e3

    xT_d = nc.dram_tensor("xT", [P, TCH * CCH * P], fp8, kind="ExternalInput")
    xN_d = nc.dram_tensor("xN", [P, CCH * TCH * P], fp8, kind="ExternalInput")
    wv_d = nc.dram_tensor("wv", [P, CCH * H], fp16, kind="ExternalInput")
    wvp_d = nc.dram_tensor("wvp", [P, CCH * CCH * P], fp8, kind="ExternalInput")
    rz_d = nc.dram_tensor("rz", [P, CCH * H], fp32, kind="ExternalInput")
    ez_d = nc.dram_tensor("ez", [1, H], fp32, kind="ExternalInput")
    wvf_d = nc.dram_tensor("wvf", [P, CCH * C], fp8, kind="ExternalInput")
    out_d = nc.dram_tensor("out", [P, CCH], fp32, kind="ExternalOutput")

    with ExitStack() as ctx:
        tc = ctx.enter_context(tile.TileContext(nc))
        big = ctx.enter_context(tc.tile_pool(name="big", bufs=1))
        psS = ctx.enter_context(tc.tile_pool(name="psS", bufs=1, space="PSUM"))
        psD = ctx.enter_context(tc.tile_pool(name="psD", bufs=1, space="PSUM"))
        psR = ctx.enter_context(tc.tile_pool(name="psR", bufs=2, space="PSUM"))
        psO = ctx.enter_context(tc.tile_pool(name="psO", bufs=1, space="PSUM"))
        psF = ctx.enter_context(tc.tile_pool(name="psF", bufs=1, space="PSUM"))

        xT_sb = big.tile([P, TCH * CCH * P], fp8, tag="xT")
        xN_sb = big.tile([P, CCH * TCH * P], fp8, tag="xN")
        wv_sb = big.tile([P, CCH * H], fp16, tag="wv")
        wvp_sb = big.tile([P, CCH * CCH * P], fp8, tag="wvp")
        wvf_sb = big.tile([P, CCH * C], fp8, tag="wvf")
        attT_sb = big.tile([P, TCH * H], fp16, tag="attT")
        rT_sb = big.tile([P, CCH * H], fp16, tag="rT")
        oT_sb = big.tile([P, CCH], fp16, tag="oT")
        ones_sb = big.tile([P, 1], fp16, tag="ones")
        rz_sb = big.tile([P, CCH * H], fp32, tag="rz")
        ez_sb = big.tile([1, H], fp32, tag="ez")
        ones1_sb = big.tile([1, P], fp16, tag="ones1")  # filled with 1/256
        den_sb = big.tile([1, H], fp32, tag="den")
        invd_sb = big.tile([1, H], fp32, tag="invd")
        invd16_sb = big.tile([1, H], fp16, tag="invd16")
        out_sb = big.tile([P, CCH], fp32, tag="out")

        def _body():
            nc.sync.dma_start(wv_sb[:], wv_d[:, :])
            for s in range(16384 // XSLAB):
                lo, hi = s * XSLAB, (s + 1) * XSLAB
                nc.sync.dma_start(xT_sb[:, lo:hi], xT_d[:, lo:hi])
                if s == 0:
                    nc.sync.dma_start(rz_sb[:], rz_d[:, :])
                    nc.sync.dma_start(ez_sb[:], ez_d[:, :])
            for s in range(16384 // XSLAB):
                lo, hi = s * XSLAB, (s + 1) * XSLAB
                nc.sync.dma_start(xN_sb[:, lo:hi], xN_d[:, lo:hi])
            for s in range(8192 // WSLAB):
                lo, hi = s * WSLAB, (s + 1) * WSLAB
                nc.sync.dma_start(wvp_sb[:, lo:hi], wvp_d[:, lo:hi])
            for s in range(8192 // WSLAB):
                lo, hi = s * WSLAB, (s + 1) * WSLAB
                nc.sync.dma_start(wvf_sb[:, lo:hi], wvf_d[:, lo:hi])

            nc.vector.memset(ones_sb[:, :], 1.0)
            ms_inst = nc.vector.memset(ones1_sb[:, :], 1.0 / (WS * WS))

            jn0 = nc.engines[mybir.EngineType.PE].nop(nofuse=True, hint="join_start")
            add_dep_helper(jn0.ins, ms_inst.ins, reason="absorb preamble DVE ticks")
            pst = psS.tile([P, TCH * H], fp32, tag="s")
            for t in range(TCH):
                for j in range(CCH):
                    nc.tensor.matmul(
                        pst[:, H * t : H * (t + 1)],
                        xT_sb[:, (t * CCH + j) * P : (t * CCH + j + 1) * P],
                        wv_sb[:, H * j : H * (j + 1)],
                        start=(j == 0),
                        stop=(j == CCH - 1),
                    )

            exp_inst = nc.scalar.activation(
                attT_sb[:, :], pst[:, :], mybir.ActivationFunctionType.Exp
            )

            jn = nc.engines[mybir.EngineType.PE].nop(nofuse=True, hint="join_den")
            add_dep_helper(jn.ins, exp_inst.ins, reason="join exp->denominator")
            pd = psD.tile([1, H], fp32, tag="d")
            for t in range(TCH):
                nc.tensor.matmul(
                    pd[:, :],
                    ones_sb[:, :],
                    attT_sb[:, H * t : H * (t + 1)],
                    start=(t == 0),
                    stop=(t == TCH - 1),
                )
            nc.vector.tensor_tensor(
                den_sb[0:1, :], pd[0:1, :], ez_sb[0:1, :], mybir.AluOpType.add
            )
            nc.vector.reciprocal(invd_sb[0:1, :], den_sb[0:1, :])
            nc.vector.tensor_copy(invd16_sb[0:1, :], invd_sb[0:1, :])
            pbc = psD.tile([P, H], fp32, tag="bc")
            nc.tensor.matmul(
                pbc[:, :], ones1_sb[:, :], invd16_sb[0:1, :], start=True, stop=True
            )

            for j in range(CCH):
                pr = psR.tile([P, H], fp32, tag="r")
                for t in range(TCH):
                    nc.tensor.matmul(
                        pr[:, :],
                        xN_sb[:, (j * TCH + t) * P : (j * TCH + t + 1) * P],
                        attT_sb[:, H * t : H * (t + 1)],
                        start=(t == 0),
                        stop=(t == TCH - 1),
                    )
                nc.vector.tensor_tensor(
                    rT_sb[:, H * j : H * (j + 1)], pr[:, :],
                    rz_sb[:, H * j : H * (j + 1)], mybir.AluOpType.add,
                )
            scale_inst = nc.vector.tensor_tensor(
                rT_sb[:, :].rearrange("p (j h) -> p j h", h=H),
                rT_sb[:, :].rearrange("p (j h) -> p j h", h=H),
                pbc[:, None, :].to_broadcast((P, CCH, H)),
                mybir.AluOpType.mult,
            )
            jn2 = nc.engines[mybir.EngineType.PE].nop(nofuse=True, hint="join_o")
            add_dep_helper(jn2.ins, scale_inst.ins, reason="join rT scale -> o matmuls")

            po = psO.tile([P, CCH], fp32, tag="o")
            for j in range(CCH):
                for jj in range(CCH):
                    nc.tensor.matmul(
                        po[:, j : j + 1],
                        wvp_sb[:, (j * CCH + jj) * P : (j * CCH + jj + 1) * P],
                        rT_sb[:, H * jj + j // 2 : H * jj + j // 2 + 1],
                        start=(jj == 0),
                        stop=(jj == CCH - 1),
                    )
                ot_copy = nc.vector.tensor_copy(oT_sb[:, j : j + 1], po[:, j : j + 1])

            jn3 = nc.engines[mybir.EngineType.PE].nop(nofuse=True, hint="join_f")
            add_dep_helper(jn3.ins, ot_copy.ins, reason="join oT copies -> f matmuls")
            pf = psF.tile([P, CCH], fp32, tag="f")
            for jo in range(CCH):
                for jc in range(CCH):
                    nc.tensor.matmul(
                        pf[:, jo : jo + 1],
                        wvf_sb[:, (jo * CCH + jc) * P : (jo * CCH + jc + 1) * P],
                        oT_sb[:, jc : jc + 1],
                        start=(jc == 0),
                        stop=(jc == CCH - 1),
                    )
            nc.vector.tensor_copy(out_sb[:, :], pf[:, :])
            nc.sync.dma_start(out_d[:, :], out_sb[:, :])

        if reps == 1:
            _body()
        else:
            with tc.For_i(0, reps, 1, hint_engines=(mybir.EngineType.PE,)):
                _body()

    nc.finalize()
    return nc


def _prep_inputs(input, tmp_token, Wqkv, bqkv, Wv, bv):
    x = np.asarray(input, dtype=np.float32)
    tmp = np.asarray(tmp_token, dtype=np.float32)
    Wqkv = np.asarray(Wqkv, dtype=np.float32)
    bqkv = np.asarray(bqkv, dtype=np.float32)
    Wvf = np.asarray(Wv, dtype=np.float32)
    bvf = np.asarray(bv, dtype=np.float32)

    Wq, Wk, Wvp = Wqkv[:, :C], Wqkv[:, C : 2 * C], Wqkv[:, 2 * C :]
    bq, bvp = bqkv[:C], bqkv[2 * C :]

    q0 = tmp[0, 0] @ Wq + bq
    q0h = q0.reshape(H, Dh)
    scale = np.float32(Dh) ** -0.5
    w = np.stack(
        [Wk[:, h * Dh : (h + 1) * Dh] @ q0h[h] for h in range(H)], axis=1
    ) * scale
    cst = (bvp @ Wvf + bvf).astype(np.float32)

    wv_sb = np.ascontiguousarray(
        w.reshape(CCH, P, H).transpose(1, 0, 2)
    ).reshape(P, -1).astype(F16)
    wvp_sb = np.ascontiguousarray(
        (WS * Wvp).astype(E3).reshape(CCH, P, CCH, P).transpose(1, 2, 0, 3)
    ).reshape(P, -1)
    wvf_sb = np.ascontiguousarray(
        (WS * Wvf).astype(E3).reshape(CCH, P, CCH, P).transpose(1, 2, 0, 3)
    ).reshape(P, -1)

    s0 = tmp[0, 0] @ w
    ez = np.exp(s0).astype(np.float32)[None, :]
    r0 = tmp[0, 0][:, None] * np.exp(s0)[None, :]
    rz = np.ascontiguousarray(
        r0.reshape(CCH, P, H).transpose(1, 0, 2)
    ).reshape(P, -1).astype(np.float32)

    X5 = np.ascontiguousarray(x, dtype=E3).reshape(B, TCH, P, CCH, P)
    xT_all = np.ascontiguousarray(X5.transpose(0, 4, 1, 3, 2)).reshape(B, P, -1)
    xN_all = np.ascontiguousarray(X5.transpose(0, 2, 3, 1, 4)).reshape(B, P, -1)
    in_maps = [
        {"xT": xT_all[b], "xN": xN_all[b], "wv": wv_sb, "wvp": wvp_sb,
         "wvf": wvf_sb, "rz": rz, "ez": ez}
        for b in range(B)
    ]
    return in_maps, cst


def kernel(input, tmp_token, Wqkv, bqkv, Wv, bv, _reps=1):
    global LAST_RESULTS, _NC_CACHE
    in_maps, cst = _prep_inputs(input, tmp_token, Wqkv, bqkv, Wv, bv)
    if _reps not in _NC_CACHE:
        _NC_CACHE[_reps] = _build_bass(_reps)
    nc = _NC_CACHE[_reps]
    res = run_bass_kernel_spmd(nc, in_maps, core_ids=list(range(B)))
    LAST_RESULTS = res
    return np.stack(
        [res.results[b]["out"].T.reshape(C) + cst for b in range(B)]
    ).astype(np.float32)
